# revision 39
# baseline (speedup 1.0000x reference)
"""Multi-head causal attention on 8 Trainium2 cores.

Sharding: core = (batch b in 0..3, head-group g in 0..1). Each core computes
Q/K/V projections for its 8 heads of its batch, causal attention, and a
partial output projection (Wo row-split); host sums the two partials per
batch and transposes back.

Device layout notes (v3 — bf16 SBUF-resident, phase-overlapped):
  - All matmul inputs are bf16 (1 cyc/row on PE, same as fp32r, half SBUF).
  - Q^T, K^T, V stay resident in SBUF between projection and attention.
  - A1 projects Q/K with k-outer 8-bank accumulation so PE streams with the
    x DMA; Q and K chunk groups are staggered so PSUM copies never stall.
  - h6/h7's chunk-2,3 projections are deferred and interleaved into half-0
    attention as PE filler (covers the exp-latency ladder + keeps the PE
    pstate at max clock).
  - Softmax denominator via DVE pair-sums + one ones-matmul per head.
  - Half-0's output projection is interleaved inside half-1's attention kt
    loops; wo is loaded in nt-slices matching consumption order.
"""

import numpy as np
import ml_dtypes

import concourse.bacc as bacc
import concourse.mybir as mybir
import concourse.tile as tile
from concourse.bass_utils import run_bass_kernel_spmd

B, T, D = 4, 2048, 2048
NH, HD = 16, 128
G = 8                       # heads per core
GD = G * HD                 # 1024, group channel width
P = 128
QC = 512                    # q-chunk (PSUM bank width in fp32)
NKT = T // P                # 16 k-tiles over the sequence
NDK = D // P                # 16 k-tiles over d_in
NQC = T // QC               # 4 q-chunks
SCALE = 1.0 / float(np.sqrt(HD))
NEG = -1.0e30

F32 = mybir.dt.float32
F32R = mybir.dt.float32r
BF16 = mybir.dt.bfloat16
EXP = mybir.ActivationFunctionType.Exp


DEFER = True
A1_STREAM = True


def build_kernel(debug_dump=False):
    nc = bacc.Bacc("TRN2", target_bir_lowering=False, debug=False, num_devices=8,
                   dynamic_dma_scratch_size=2048)

    xT = nc.dram_tensor("xT", [D, T], BF16, kind="ExternalInput")
    # pre-tiled on host: wq/wk [head, p, ko, d], wv [dchunk, p, ko, c]
    wqT = nc.dram_tensor("wqT", [G, P, NDK, HD], BF16, kind="ExternalInput")
    wkT = nc.dram_tensor("wkT", [G, P, NDK, HD], BF16, kind="ExternalInput")
    wvT = nc.dram_tensor("wvT", [2, P, NDK, QC], BF16, kind="ExternalInput")
    # wo pre-tiled nt-major: [nt, p, hh, 128]
    woT = nc.dram_tensor("woT", [NDK, P, G, P], BF16, kind="ExternalInput")
    # triangle mask: NEG where partition (k) > column (q) within a 128 block
    maskadd = nc.dram_tensor("maskadd", [P, P], F32, kind="ExternalInput")
    outT = nc.dram_tensor("outT", [D, T], BF16, kind="ExternalOutput")
    if debug_dump:
        qtD = nc.dram_tensor("qtD", [P, G, T], BF16, kind="ExternalOutput")
        ktD = nc.dram_tensor("ktD", [P, G, T], BF16, kind="ExternalOutput")
        vtD = nc.dram_tensor("vtD", [P, NKT, G, HD], BF16, kind="ExternalOutput")
        c2D = nc.dram_tensor("c2D", [P, 2, G, QC], BF16, kind="ExternalOutput")
        mkD = nc.dram_tensor("mkD", [P, P], F32, kind="ExternalOutput")
        onD = nc.dram_tensor("onD", [P, P], F32, kind="ExternalOutput")

    xT_t = xT.rearrange("(ko p) t -> p ko t", p=P)
    outT_t = outT.rearrange("(no p) t -> p no t", p=P)

    with tile.TileContext(nc) as tc:
        with (
            tc.tile_pool(name="const", bufs=1) as constp,
            tc.tile_pool(name="kvq", bufs=1) as kvqp,
            tc.tile_pool(name="c2p0", bufs=1) as c2p0,
        ):
            ones_sb = constp.tile([P, P], BF16)
            nc.vector.memset(ones_sb, 1.0)
            mask_sb = constp.tile([P, P], F32)

            kt_sb = kvqp.tile([P, G, T], BF16)           # K^T per head
            qt_sb = kvqp.tile([P, G, T], BF16)           # Q^T per head
            vt_sb = kvqp.tile([P, NKT, G, HD], BF16)     # V per head
            ctx2_0 = c2p0.tile([P, 2, G, QC], BF16)      # half-0 attn output

            outq = [nc.sync, nc.sync]
            oidx = [0]

            def qk_copy(dst, h, c, ps, eng):
                if eng == 0:
                    nc.scalar.copy(dst[:, h, c * QC:(c + 1) * QC], ps)
                else:
                    nc.vector.tensor_copy(dst[:, h, c * QC:(c + 1) * QC], ps)

            with tc.tile_pool(name="xpool", bufs=1) as xpool:
                xt_sb = xpool.tile([P, NDK, T], BF16)    # 8 MB, resident

                # ---------------- A1: Q/K projections ----------------
                with (
                    tc.tile_pool(name="w1pool", bufs=2) as w1p,
                    tc.tile_pool(name="psA1", bufs=1, space="PSUM") as psA1,
                ):
                    # first head's weights before the x stream; the first
                    # few k-slices land as small DMAs so matmul k=0 can
                    # start as early as possible
                    wq_sb = w1p.tile([P, NDK, HD], BF16, tag="wq")
                    for kk in (slice(0, 2), slice(2, 8), slice(8, NDK)):
                        nc.scalar.dma_start(wq_sb[:, kk], wqT[0, :, kk])
                    wk_sb = w1p.tile([P, NDK, HD], BF16, tag="wk")
                    for kk in (slice(0, 2), slice(2, 8), slice(8, NDK)):
                        nc.scalar.dma_start(wk_sb[:, kk], wkT[0, :, kk])
                    nc.scalar.dma_start(mask_sb, maskadd[:])

                    for k in range(NDK):
                        if k < 2:
                            for cc in range(4):
                                nc.sync.dma_start(
                                    xt_sb[:, k, cc * QC:(cc + 1) * QC],
                                    xT_t[:, k, cc * QC:(cc + 1) * QC])
                        else:
                            nc.sync.dma_start(xt_sb[:, k], xT_t[:, k])

                    for h in range(G):
                        full = h < 6
                        if h > 0:
                            wq_sb = w1p.tile([P, NDK, HD], BF16, tag="wq")
                            nc.scalar.dma_start(wq_sb, wqT[h])
                            wk_sb = w1p.tile([P, NDK, HD], BF16, tag="wk")
                            nc.scalar.dma_start(wk_sb, wkT[h])
                        # group 1: Q all chunks (c01 for h6/h7) + K c0,c1
                        qcs = (0, 1, 2, 3) if full else (0, 1)
                        kcs = (0, 1)
                        psq = {c: psA1.tile([P, QC], F32, tag=f"q{c}", name=f"psq{c}")
                               for c in qcs}
                        psk = {c: psA1.tile([P, QC], F32, tag=f"k{c}", name=f"psk{c}")
                               for c in kcs}
                        for k in range(NDK):
                            st, sp = (k == 0), (k == NDK - 1)
                            for c in qcs:
                                nc.tensor.matmul(
                                    psq[c], wq_sb[:, k],
                                    xt_sb[:, k, c * QC:(c + 1) * QC],
                                    start=st, stop=sp)
                            for c in kcs:
                                nc.tensor.matmul(
                                    psk[c], wk_sb[:, k],
                                    xt_sb[:, k, c * QC:(c + 1) * QC],
                                    start=st, stop=sp)
                        for i, c in enumerate(qcs):
                            qk_copy(qt_sb, h, c, psq[c], i % 2)
                        for i, c in enumerate(kcs):
                            qk_copy(kt_sb, h, c, psk[c], (i + 1) % 2)
                        # group 2: K c2,c3 (full heads) — drains while group-1
                        # copies free their banks
                        if full:
                            psk2 = {c: psA1.tile([P, QC], F32, tag=f"k{c}", name=f"psk2{c}")
                                    for c in (2, 3)}
                            for k in range(NDK):
                                for c in (2, 3):
                                    nc.tensor.matmul(
                                        psk2[c], wk_sb[:, k],
                                        xt_sb[:, k, c * QC:(c + 1) * QC],
                                        start=(k == 0), stop=(k == NDK - 1))
                            qk_copy(kt_sb, h, 2, psk2[2], 0)
                            qk_copy(kt_sb, h, 3, psk2[3], 1)

                # ---------------- A2: V projection (dc-split) ----------------
                # wv streams per-k so the V k-loop starts early; two deferred
                # h6 c2/c3 projections fill the PE while the first slices land
                for dc in range(2):
                    with (
                        tc.tile_pool(name=f"wv{dc}", bufs=1) as wvp,
                        tc.tile_pool(name=f"wA2{dc}", bufs=1) as wA2p,
                        tc.tile_pool(name=f"psV{dc}", bufs=2,
                                     space="PSUM") as psV,
                    ):
                        wv_sb = wvp.tile([P, NDK, QC], BF16)
                        for k0 in range(0, NDK, 4):
                            nc.scalar.dma_start(wv_sb[:, k0:k0 + 4],
                                                wvT[dc, :, k0:k0 + 4])
                        wA2 = wA2p.tile([P, NDK, HD], BF16)
                        nc.sync.dma_start(wA2, (wqT if dc == 0 else wkT)[6])
                        for c in (2, 3):
                            ps = psV.tile([P, QC], F32, tag="def", bufs=1)
                            for k in range(NDK):
                                nc.tensor.matmul(
                                    ps, wA2[:, k],
                                    xt_sb[:, k, c * QC:(c + 1) * QC],
                                    start=(k == 0), stop=(k == NDK - 1))
                            dst = qt_sb if dc == 0 else kt_sb
                            nc.vector.tensor_copy(
                                dst[:, 6, c * QC:(c + 1) * QC], ps)
                        for ts in range(NKT):
                            ps = psV.tile([P, QC], F32, tag="v")
                            for k in range(NDK):
                                nc.tensor.matmul(
                                    ps, xt_sb[:, k, ts * P:(ts + 1) * P],
                                    wv_sb[:, k],
                                    start=(k == 0), stop=(k == NDK - 1))
                            nc.vector.tensor_copy(
                                vt_sb[:, ts, 4 * dc:4 * (dc + 1), :],
                                ps.rearrange("p (g c) -> p g c", g=4))

                # ---------------- overlap: half-0 attention + deferred
                # c2/c3 projections of h6/h7 as PE filler ----------------
                with (
                    tc.tile_pool(name="w2pool", bufs=2) as w2p,
                    tc.tile_pool(name="pp0", bufs=4) as pp0,
                    tc.tile_pool(name="prp0", bufs=2) as prp0,
                    tc.tile_pool(name="accp0", bufs=1) as accp0,
                    tc.tile_pool(name="izp0", bufs=1) as izp0,
                    tc.tile_pool(name="psS0", bufs=2, space="PSUM") as psS0,
                    tc.tile_pool(name="psC0", bufs=1, space="PSUM") as psC0,
                    tc.tile_pool(name="psZD", bufs=1, space="PSUM") as psZD,
                ):
                    # deferred unit list: grouped so one w tile serves 2 units
                    defer = [(wt, 7, c) for wt in (0, 1) for c in (2, 3)]
                    dstate = {"i": 0, "w": None}

                    def defer_w_load(gi):
                        wt, h, _ = defer[2 * gi]
                        w = w2p.tile([P, NDK, HD], BF16, tag="w2")
                        nc.scalar.dma_start(w, (wqT if wt == 0 else wkT)[h])
                        return w

                    dstate["w"] = defer_w_load(0)
                    dstate["wnext"] = None

                    def filler0():
                        i = dstate["i"]
                        if i >= len(defer):
                            return
                        wt, h, c = defer[i]
                        if i % 2 == 0 and i + 2 < len(defer):
                            dstate["wnext"] = defer_w_load((i + 2) // 2)
                        w_sb = dstate["w"]
                        ps = psZD.tile([P, QC], F32, tag="def")
                        for k in range(NDK):
                            nc.tensor.matmul(
                                ps, w_sb[:, k],
                                xt_sb[:, k, c * QC:(c + 1) * QC],
                                start=(k == 0), stop=(k == NDK - 1))
                        dst = qt_sb if wt == 0 else kt_sb
                        nc.vector.tensor_copy(
                            dst[:, h, c * QC:(c + 1) * QC], ps)
                        if i % 2 == 1:
                            dstate["w"] = dstate["wnext"]
                        dstate["i"] = i + 1

                    for h in range(G):
                        attn_head(nc, h, (0, 1), ctx2_0, psC0, psS0, psZD,
                                  pp0, prp0, accp0, izp0,
                                  kt_sb, qt_sb, vt_sb, ones_sb, mask_sb,
                                  filler=filler0,
                                  fill_points=(1,) if h % 2 == 0 else ())

            if debug_dump:
                nc.sync.dma_start(mkD[:], mask_sb)
                nc.sync.dma_start(onD[:], ones_sb.bitcast(F32))
                for h_ in range(G):
                    nc.sync.dma_start(qtD[:, h_], qt_sb[:, h_])
                    nc.sync.dma_start(ktD[:, h_], kt_sb[:, h_])
                    nc.sync.dma_start(c2D[:, 0, h_], ctx2_0[:, 0, h_])
                    nc.sync.dma_start(c2D[:, 1, h_], ctx2_0[:, 1, h_])
                for ts_ in range(NKT):
                    nc.gpsimd.dma_start(vtD[:, ts_], vt_sb[:, ts_])

            # x freed; half-1 attention + both output projections
            with (
                tc.tile_pool(name="wopool", bufs=1) as wop,
                tc.tile_pool(name="c2p1", bufs=1) as c2p1,
                tc.tile_pool(name="pp1", bufs=4) as pp1,
                tc.tile_pool(name="prp1", bufs=2) as prp1,
                tc.tile_pool(name="accp1", bufs=1) as accp1,
                tc.tile_pool(name="izp1", bufs=1) as izp1,
                tc.tile_pool(name="opool", bufs=3) as op_,
                tc.tile_pool(name="psS1", bufs=2, space="PSUM") as psS1,
                tc.tile_pool(name="psC1", bufs=1, space="PSUM") as psC1,
                tc.tile_pool(name="psZO", bufs=1, space="PSUM") as psZO,
            ):
                ctx2_1 = c2p1.tile([P, 2, G, QC], BF16)
                wo_sb = wop.tile([P, NDK, G, P], BF16)   # 4 MB, nt-major
                for nt in range(NDK):
                    outq[nt % 2].dma_start(wo_sb[:, nt], woT[nt])

                ostate = {"i": 0}
                otiles = [(nt, ci, ci, ctx2_0) for nt in range(NDK)
                          for ci in range(2)]

                def outproj_tile(nt, ci, c, ctx2src, final=False):
                    if final:
                        o_ps = psC1.tile([P, QC], F32,
                                         tag=f"ctx{oidx[0] % 2}",
                                         name="o_ps")
                    else:
                        o_ps = psZO.tile([P, QC], F32, tag="o")
                    for hh in range(G):
                        nc.tensor.matmul(
                            o_ps, wo_sb[:, nt, hh], ctx2src[:, ci, hh],
                            start=(hh == 0), stop=(hh == G - 1))
                    o_sb = op_.tile([P, QC], BF16, tag="osb")
                    if oidx[0] % 2 == 0:
                        nc.scalar.copy(o_sb, o_ps)
                    else:
                        nc.vector.tensor_copy(o_sb, o_ps)
                    nc.sync.dma_start(
                        outT_t[:, nt, c * QC:(c + 1) * QC], o_sb)
                    oidx[0] += 1

                def filler1():
                    i = ostate["i"]
                    if i >= len(otiles):
                        return
                    outproj_tile(*otiles[i])
                    ostate["i"] = i + 1

                # two outproj tiles up front to cover the phase
                # transition before head 0's exp ladder warms up
                filler1()
                filler1()
                for h in range(G):
                    attn_head(nc, h, (2, 3), ctx2_1, psC1, psS1, psZO,
                              pp1, prp1, accp1, izp1,
                              kt_sb, qt_sb, vt_sb, ones_sb, mask_sb,
                              filler=filler1, fill_points=(0, 1, 2, 3))
                for nt in range(NDK):
                    for ci in range(2):
                        outproj_tile(nt, ci, 2 + ci, ctx2_1, final=True)

    nc.finalize()
    return nc


def attn_head(nc, h, c_pair, ctx2, psC, psS, psZ, pp, prp, accp, izp,
              kt_sb, qt_sb, vt_sb, ones_sb, mask_sb,
              filler=None, fill_points=()):
    """Causal attention for head h over q-chunks c_pair.

    Softmax denominator: exp tiles pair-summed on DVE into a bf16
    accumulator; one ones-matmul per (head, chunk) broadcasts the
    partition-sum. filler() emits one independent PE work unit at up to 4
    insertion points to bridge exp-latency stalls.
    """
    def fill(point):
        if filler is not None and point in fill_points:
            filler()

    accs, ctxps = [], []
    for ci, c in enumerate(c_pair):
        acc = accp.tile([P, QC], BF16, tag=f"acc{ci}")
        ctx_ps = psC.tile([P, QC], F32, tag=f"ctx{ci}")
        qs = qt_sb[:, h, c * QC:(c + 1) * QC]
        nd = 4 * c  # number of full (non-diagonal) k-tiles
        for kt2 in range(0, nd, 2):
            # one 2-bank PSUM tile + one wide exp per pair: halves the exp
            # count and the ladder semaphore hops
            s2 = psS.tile([P, 2 * QC], F32, tag="s")
            nc.tensor.matmul(s2[:, 0:QC],
                             kt_sb[:, h, kt2 * P:(kt2 + 1) * P], qs,
                             start=True, stop=True)
            nc.tensor.matmul(s2[:, QC:2 * QC],
                             kt_sb[:, h, (kt2 + 1) * P:(kt2 + 2) * P],
                             qs, start=True, stop=True)
            p2 = pp.tile([P, 2 * QC], BF16, tag="p")
            nc.scalar.activation(p2, s2, EXP, scale=SCALE)
            if kt2 == 0:
                nc.vector.tensor_add(acc, p2[:, 0:QC], p2[:, QC:2 * QC])
            else:
                pr = prp.tile([P, QC], BF16, tag="pr")
                nc.vector.tensor_add(pr, p2[:, 0:QC], p2[:, QC:2 * QC])
                nc.vector.tensor_add(acc, acc, pr)
            nc.tensor.matmul(ctx_ps, vt_sb[:, kt2, h], p2[:, 0:QC],
                             start=(kt2 == 0), stop=False)
            nc.tensor.matmul(ctx_ps, vt_sb[:, kt2 + 1, h], p2[:, QC:2 * QC],
                             start=False, stop=False)
        if ci == 1:
            fill(1)
        for j in range(4):
            F = QC - j * P
            kt = nd + j
            s1 = psS.tile([P, 2 * QC], F32, tag="s", name="s1")
            nc.tensor.matmul(s1[:, 0:F], kt_sb[:, h, kt * P:(kt + 1) * P],
                             qs[:, j * P:QC], start=True, stop=True)
            nc.vector.tensor_add(s1[:, 0:P], s1[:, 0:P], mask_sb)
            p1 = pp.tile([P, 2 * QC], BF16, tag="p")
            nc.scalar.activation(p1[:, 0:F], s1[:, 0:F], EXP, scale=SCALE)
            if j == 0 and nd == 0:
                nc.vector.tensor_copy(acc, p1[:, 0:QC])
            else:
                nc.vector.tensor_add(acc[:, j * P:QC], acc[:, j * P:QC],
                                     p1[:, 0:F])
            nc.tensor.matmul(ctx_ps[:, j * P:QC], vt_sb[:, kt, h], p1[:, 0:F],
                             start=(nd == 0 and j == 0), stop=(j == 3))
        accs.append(acc)
        ctxps.append(ctx_ps)
        fill(0 if ci == 0 else 2)
    for ci in range(2):
        zw = psZ.tile([P, QC], F32, tag="z")
        nc.tensor.matmul(zw, ones_sb, accs[ci], start=True, stop=True)
        iz = izp.tile([P, QC], F32, tag=f"iz{ci}")
        nc.vector.reciprocal_approx_fast(iz, zw)
        nc.vector.tensor_mul(ctx2[:, ci, h], ctxps[ci], iz)
    fill(3)


_NC = None
DEBUG_NC = False


def _get_nc():
    global _NC
    if _NC is None:
        _NC = build_kernel(debug_dump=DEBUG_NC)
    return _NC


def _make_mask():
    m = np.zeros((P, P), dtype=np.float32)
    i = np.arange(P)[:, None]
    col = np.arange(P)[None, :]
    m[i > col] = NEG
    return m


def kernel(x, Wq, Wk, Wv, Wo, _trace=False, _trace_kwargs=None):
    bf16 = ml_dtypes.bfloat16
    x = np.asarray(x, dtype=np.float32)
    Wq = np.asarray(Wq, dtype=np.float32)
    Wk = np.asarray(Wk, dtype=np.float32)
    Wv = np.asarray(Wv, dtype=np.float32)
    Wo = np.asarray(Wo, dtype=np.float32)

    nc = _get_nc()
    mask = _make_mask()

    # [d_out, d_in] -> [h, p, ko, dd] tiles per head-group chunk of 8 heads
    def tile_qk(W, g):
        wt = W.T[:, g * GD:(g + 1) * GD]              # [D, GD]
        return np.ascontiguousarray(
            wt.reshape(NDK, P, G, HD).transpose(2, 1, 0, 3).astype(bf16))

    def tile_v(W, g):
        wt = W.T[:, g * GD:(g + 1) * GD]              # [D, GD]
        return np.ascontiguousarray(
            wt.reshape(NDK, P, 2, QC).transpose(2, 1, 0, 3).astype(bf16))

    def tile_wo(W, g):
        wt = W.T[g * GD:(g + 1) * GD, :]              # [GD, D]
        # [nt, p(of head block), hh, 128]
        return np.ascontiguousarray(
            wt.reshape(G, P, NDK, P).transpose(2, 1, 0, 3).astype(bf16))

    in_maps = []
    for core in range(8):
        b, g = divmod(core, 2)
        in_maps.append({
            "xT": np.ascontiguousarray(x[b].T.astype(bf16)),
            "wqT": tile_qk(Wq, g),
            "wkT": tile_qk(Wk, g),
            "wvT": tile_v(Wv, g),
            "woT": tile_wo(Wo, g),
            "maskadd": mask,
        })

    kwargs = {}
    if _trace:
        kwargs.update(trace=True, **(_trace_kwargs or {}))
    res = run_bass_kernel_spmd(nc, in_maps, core_ids=list(range(8)), **kwargs)

    out = np.empty((B, T, D), dtype=np.float32)
    for b in range(B):
        acc = (np.asarray(res.results[2 * b]["outT"], dtype=np.float32)
               + np.asarray(res.results[2 * b + 1]["outT"], dtype=np.float32))
        out[b] = acc.T
    if _trace:
        return out, res
    return out


# revision 40
# speedup vs baseline: 1.0567x; 1.0567x over previous
"""Multi-head causal attention on 8 Trainium2 cores.

Sharding: core = (batch b in 0..3, head-group g in 0..1). Each core computes
Q/K/V projections for its 8 heads of its batch, causal attention, and a
partial output projection (Wo row-split); host sums the two partials per
batch and transposes back.

Device layout notes (v3 — bf16 SBUF-resident, phase-overlapped):
  - All matmul inputs are bf16 (1 cyc/row on PE, same as fp32r, half SBUF).
  - Q^T, K^T, V stay resident in SBUF between projection and attention.
  - A1 projects Q/K with k-outer 8-bank accumulation so PE streams with the
    x DMA; Q and K chunk groups are staggered so PSUM copies never stall.
  - h6/h7's chunk-2,3 projections are deferred and interleaved into half-0
    attention as PE filler (covers the exp-latency ladder + keeps the PE
    pstate at max clock).
  - Softmax denominator via DVE pair-sums + one ones-matmul per head.
  - Half-0's output projection is interleaved inside half-1's attention kt
    loops; wo is loaded in nt-slices matching consumption order.
"""

import numpy as np
import ml_dtypes

import concourse.bacc as bacc
import concourse.mybir as mybir
import concourse.tile as tile
from concourse.bass_utils import run_bass_kernel_spmd

B, T, D = 4, 2048, 2048
NH, HD = 16, 128
G = 8                       # heads per core
GD = G * HD                 # 1024, group channel width
P = 128
QC = 512                    # q-chunk (PSUM bank width in fp32)
NKT = T // P                # 16 k-tiles over the sequence
NDK = D // P                # 16 k-tiles over d_in
NQC = T // QC               # 4 q-chunks
SCALE = 1.0 / float(np.sqrt(HD))
NEG = -1.0e30

F32 = mybir.dt.float32
F32R = mybir.dt.float32r
BF16 = mybir.dt.bfloat16
EXP = mybir.ActivationFunctionType.Exp


DEFER = True
A1_STREAM = True


def build_kernel(debug_dump=False):
    nc = bacc.Bacc("TRN2", target_bir_lowering=False, debug=False, num_devices=8,
                   dynamic_dma_scratch_size=2048)

    xT = nc.dram_tensor("xT", [D, T], BF16, kind="ExternalInput")
    # pre-tiled on host: wq/wk [head, p, ko, d], wv [dchunk, p, ko, c]
    wqT = nc.dram_tensor("wqT", [G, P, NDK, HD], BF16, kind="ExternalInput")
    wkT = nc.dram_tensor("wkT", [G, P, NDK, HD], BF16, kind="ExternalInput")
    wvT = nc.dram_tensor("wvT", [2, P, NDK, QC], BF16, kind="ExternalInput")
    # wo pre-tiled nt-major: [nt, p, hh, 128]
    woT = nc.dram_tensor("woT", [NDK, P, G, P], BF16, kind="ExternalInput")
    # triangle mask: NEG where partition (k) > column (q) within a 128 block
    maskadd = nc.dram_tensor("maskadd", [P, P], F32, kind="ExternalInput")
    outT = nc.dram_tensor("outT", [D, T], BF16, kind="ExternalOutput")
    if debug_dump:
        qtD = nc.dram_tensor("qtD", [P, G, T], BF16, kind="ExternalOutput")
        ktD = nc.dram_tensor("ktD", [P, G, T], BF16, kind="ExternalOutput")
        vtD = nc.dram_tensor("vtD", [P, NKT, G, HD], BF16, kind="ExternalOutput")
        c2D = nc.dram_tensor("c2D", [P, 2, G, QC], BF16, kind="ExternalOutput")
        mkD = nc.dram_tensor("mkD", [P, P], F32, kind="ExternalOutput")
        onD = nc.dram_tensor("onD", [P, P], F32, kind="ExternalOutput")

    xT_t = xT.rearrange("(ko p) t -> p ko t", p=P)
    outT_t = outT.rearrange("(no p) t -> p no t", p=P)

    with tile.TileContext(nc) as tc:
        with (
            tc.tile_pool(name="const", bufs=1) as constp,
            tc.tile_pool(name="kvq", bufs=1) as kvqp,
            tc.tile_pool(name="c2p0", bufs=1) as c2p0,
        ):
            ones_sb = constp.tile([P, P], BF16)
            nc.vector.memset(ones_sb, 1.0)
            mask_sb = constp.tile([P, P], F32)

            kt_sb = kvqp.tile([P, G, T], BF16)           # K^T per head
            qt_sb = kvqp.tile([P, G, T], BF16)           # Q^T per head
            vt_sb = kvqp.tile([P, NKT, G, HD], BF16)     # V per head
            ctx2_0 = c2p0.tile([P, 2, G, QC], BF16)      # half-0 attn output

            outq = [nc.sync, nc.sync]
            oidx = [0]

            def qk_copy(dst, h, c, ps, eng):
                if eng == 0:
                    nc.scalar.copy(dst[:, h, c * QC:(c + 1) * QC], ps)
                else:
                    nc.vector.tensor_copy(dst[:, h, c * QC:(c + 1) * QC], ps)

            with tc.tile_pool(name="xpool", bufs=1) as xpool:
                xt_sb = xpool.tile([P, NDK, T], BF16)    # 8 MB, resident

                # ---------------- A1: Q/K projections ----------------
                with (
                    tc.tile_pool(name="w1pool", bufs=2) as w1p,
                    tc.tile_pool(name="psA1", bufs=1, space="PSUM") as psA1,
                ):
                    # first head's weights before the x stream; the first
                    # few k-slices land as small DMAs so matmul k=0 can
                    # start as early as possible
                    wq_sb = w1p.tile([P, NDK, HD], BF16, tag="wq")
                    for kk in (slice(0, 2), slice(2, 8), slice(8, NDK)):
                        nc.scalar.dma_start(wq_sb[:, kk], wqT[0, :, kk])
                    wk_sb = w1p.tile([P, NDK, HD], BF16, tag="wk")
                    for kk in (slice(0, 2), slice(2, 8), slice(8, NDK)):
                        nc.scalar.dma_start(wk_sb[:, kk], wkT[0, :, kk])
                    nc.scalar.dma_start(mask_sb, maskadd[:])

                    for k in range(NDK):
                        if k < 2:
                            for cc in range(4):
                                nc.sync.dma_start(
                                    xt_sb[:, k, cc * QC:(cc + 1) * QC],
                                    xT_t[:, k, cc * QC:(cc + 1) * QC])
                        else:
                            nc.sync.dma_start(xt_sb[:, k], xT_t[:, k])

                    for h in range(G):
                        full = h < 6
                        if h > 0:
                            wq_sb = w1p.tile([P, NDK, HD], BF16, tag="wq")
                            nc.scalar.dma_start(wq_sb, wqT[h])
                            wk_sb = w1p.tile([P, NDK, HD], BF16, tag="wk")
                            nc.scalar.dma_start(wk_sb, wkT[h])
                        # group 1: Q all chunks (c01 for h6/h7) + K c0,c1
                        qcs = (0, 1, 2, 3) if full else (0, 1)
                        kcs = (0, 1)
                        psq = {c: psA1.tile([P, QC], F32, tag=f"q{c}", name=f"psq{c}")
                               for c in qcs}
                        psk = {c: psA1.tile([P, QC], F32, tag=f"k{c}", name=f"psk{c}")
                               for c in kcs}
                        for k in range(NDK):
                            st, sp = (k == 0), (k == NDK - 1)
                            for c in qcs:
                                nc.tensor.matmul(
                                    psq[c], wq_sb[:, k],
                                    xt_sb[:, k, c * QC:(c + 1) * QC],
                                    start=st, stop=sp)
                            for c in kcs:
                                nc.tensor.matmul(
                                    psk[c], wk_sb[:, k],
                                    xt_sb[:, k, c * QC:(c + 1) * QC],
                                    start=st, stop=sp)
                        for i, c in enumerate(qcs):
                            qk_copy(qt_sb, h, c, psq[c], i % 2)
                        for i, c in enumerate(kcs):
                            qk_copy(kt_sb, h, c, psk[c], (i + 1) % 2)
                        # group 2: K c2,c3 (full heads) — drains while group-1
                        # copies free their banks
                        if full:
                            psk2 = {c: psA1.tile([P, QC], F32, tag=f"k{c}", name=f"psk2{c}")
                                    for c in (2, 3)}
                            for k in range(NDK):
                                for c in (2, 3):
                                    nc.tensor.matmul(
                                        psk2[c], wk_sb[:, k],
                                        xt_sb[:, k, c * QC:(c + 1) * QC],
                                        start=(k == 0), stop=(k == NDK - 1))
                            qk_copy(kt_sb, h, 2, psk2[2], 0)
                            qk_copy(kt_sb, h, 3, psk2[3], 1)

                # ---------------- A2: V projection (dc-split) ----------------
                # wv streams per-k so the V k-loop starts early; two deferred
                # h6 c2/c3 projections fill the PE while the first slices land
                for dc in range(2):
                    with (
                        tc.tile_pool(name=f"wv{dc}", bufs=1) as wvp,
                        tc.tile_pool(name=f"wA2{dc}", bufs=1) as wA2p,
                        tc.tile_pool(name=f"psV{dc}", bufs=2,
                                     space="PSUM") as psV,
                    ):
                        wv_sb = wvp.tile([P, NDK, QC], BF16)
                        for k0 in range(0, NDK, 4):
                            nc.scalar.dma_start(wv_sb[:, k0:k0 + 4],
                                                wvT[dc, :, k0:k0 + 4])
                        wA2 = wA2p.tile([P, NDK, HD], BF16)
                        nc.sync.dma_start(wA2, (wqT if dc == 0 else wkT)[6])
                        for c in (2, 3):
                            ps = psV.tile([P, QC], F32, tag="def", bufs=1)
                            for k in range(NDK):
                                nc.tensor.matmul(
                                    ps, wA2[:, k],
                                    xt_sb[:, k, c * QC:(c + 1) * QC],
                                    start=(k == 0), stop=(k == NDK - 1))
                            dst = qt_sb if dc == 0 else kt_sb
                            nc.vector.tensor_copy(
                                dst[:, 6, c * QC:(c + 1) * QC], ps)
                        for ts in range(NKT):
                            ps = psV.tile([P, QC], F32, tag="v")
                            for k in range(NDK):
                                nc.tensor.matmul(
                                    ps, xt_sb[:, k, ts * P:(ts + 1) * P],
                                    wv_sb[:, k],
                                    start=(k == 0), stop=(k == NDK - 1))
                            nc.vector.tensor_copy(
                                vt_sb[:, ts, 4 * dc:4 * (dc + 1), :],
                                ps.rearrange("p (g c) -> p g c", g=4))

                # ---------------- overlap: half-0 attention + deferred
                # c2/c3 projections of h6/h7 as PE filler ----------------
                with (
                    tc.tile_pool(name="w2pool", bufs=2) as w2p,
                    tc.tile_pool(name="pp0", bufs=6) as pp0,
                    tc.tile_pool(name="prp0", bufs=2) as prp0,
                    tc.tile_pool(name="accp0", bufs=1) as accp0,
                    tc.tile_pool(name="izp0", bufs=1) as izp0,
                    tc.tile_pool(name="psS0", bufs=4, space="PSUM") as psS0,
                    tc.tile_pool(name="psC0", bufs=1, space="PSUM") as psC0,
                    tc.tile_pool(name="psZD", bufs=1, space="PSUM") as psZD,
                ):
                    # deferred unit list: grouped so one w tile serves 2 units
                    defer = [(wt, 7, c) for wt in (0, 1) for c in (2, 3)]
                    dstate = {"i": 0, "w": None}

                    def defer_w_load(gi):
                        wt, h, _ = defer[2 * gi]
                        w = w2p.tile([P, NDK, HD], BF16, tag="w2")
                        nc.scalar.dma_start(w, (wqT if wt == 0 else wkT)[h])
                        return w

                    dstate["w"] = defer_w_load(0)
                    dstate["wnext"] = None

                    def filler0():
                        i = dstate["i"]
                        if i >= len(defer):
                            return
                        wt, h, c = defer[i]
                        if i % 2 == 0 and i + 2 < len(defer):
                            dstate["wnext"] = defer_w_load((i + 2) // 2)
                        w_sb = dstate["w"]
                        ps = psZD.tile([P, QC], F32, tag="def")
                        for k in range(NDK):
                            nc.tensor.matmul(
                                ps, w_sb[:, k],
                                xt_sb[:, k, c * QC:(c + 1) * QC],
                                start=(k == 0), stop=(k == NDK - 1))
                        dst = qt_sb if wt == 0 else kt_sb
                        nc.vector.tensor_copy(
                            dst[:, h, c * QC:(c + 1) * QC], ps)
                        if i % 2 == 1:
                            dstate["w"] = dstate["wnext"]
                        dstate["i"] = i + 1

                    for h in range(G):
                        attn_head(nc, h, (0, 1), ctx2_0, psC0, psS0, psZD,
                                  pp0, prp0, accp0, izp0,
                                  kt_sb, qt_sb, vt_sb, ones_sb, mask_sb,
                                  filler=filler0,
                                  fill_points=(1,) if h % 2 == 0 else ())

            if debug_dump:
                nc.sync.dma_start(mkD[:], mask_sb)
                nc.sync.dma_start(onD[:], ones_sb.bitcast(F32))
                for h_ in range(G):
                    nc.sync.dma_start(qtD[:, h_], qt_sb[:, h_])
                    nc.sync.dma_start(ktD[:, h_], kt_sb[:, h_])
                    nc.sync.dma_start(c2D[:, 0, h_], ctx2_0[:, 0, h_])
                    nc.sync.dma_start(c2D[:, 1, h_], ctx2_0[:, 1, h_])
                for ts_ in range(NKT):
                    nc.gpsimd.dma_start(vtD[:, ts_], vt_sb[:, ts_])

            # x freed; half-1 attention + both output projections
            with (
                tc.tile_pool(name="wopool", bufs=1) as wop,
                tc.tile_pool(name="c2p1", bufs=1) as c2p1,
                tc.tile_pool(name="pp1", bufs=6) as pp1,
                tc.tile_pool(name="prp1", bufs=2) as prp1,
                tc.tile_pool(name="accp1", bufs=1) as accp1,
                tc.tile_pool(name="izp1", bufs=1) as izp1,
                tc.tile_pool(name="opool", bufs=3) as op_,
                tc.tile_pool(name="psS1", bufs=4, space="PSUM") as psS1,
                tc.tile_pool(name="psC1", bufs=1, space="PSUM") as psC1,
                tc.tile_pool(name="psZO", bufs=1, space="PSUM") as psZO,
            ):
                ctx2_1 = c2p1.tile([P, 2, G, QC], BF16)
                wo_sb = wop.tile([P, NDK, G, P], BF16)   # 4 MB, nt-major
                for nt in range(NDK):
                    outq[nt % 2].dma_start(wo_sb[:, nt], woT[nt])

                ostate = {"i": 0}
                otiles = [(nt, ci, ci, ctx2_0) for nt in range(NDK)
                          for ci in range(2)]

                def outproj_tile(nt, ci, c, ctx2src, final=False):
                    if final:
                        o_ps = psC1.tile([P, QC], F32,
                                         tag=f"ctx{oidx[0] % 2}",
                                         name="o_ps")
                    else:
                        o_ps = psZO.tile([P, QC], F32, tag="o")
                    for hh in range(G):
                        nc.tensor.matmul(
                            o_ps, wo_sb[:, nt, hh], ctx2src[:, ci, hh],
                            start=(hh == 0), stop=(hh == G - 1))
                    o_sb = op_.tile([P, QC], BF16, tag="osb")
                    if oidx[0] % 2 == 0:
                        nc.scalar.copy(o_sb, o_ps)
                    else:
                        nc.vector.tensor_copy(o_sb, o_ps)
                    nc.sync.dma_start(
                        outT_t[:, nt, c * QC:(c + 1) * QC], o_sb)
                    oidx[0] += 1

                def filler1():
                    i = ostate["i"]
                    if i >= len(otiles):
                        return
                    outproj_tile(*otiles[i])
                    ostate["i"] = i + 1

                # two outproj tiles up front to cover the phase
                # transition before head 0's exp ladder warms up
                filler1()
                filler1()
                for h in range(G):
                    attn_head(nc, h, (2, 3), ctx2_1, psC1, psS1, psZO,
                              pp1, prp1, accp1, izp1,
                              kt_sb, qt_sb, vt_sb, ones_sb, mask_sb,
                              filler=filler1, fill_points=(0, 1, 2, 3))
                for nt in range(NDK):
                    for ci in range(2):
                        outproj_tile(nt, ci, 2 + ci, ctx2_1, final=True)

    nc.finalize()
    return nc


def attn_head(nc, h, c_pair, ctx2, psC, psS, psZ, pp, prp, accp, izp,
              kt_sb, qt_sb, vt_sb, ones_sb, mask_sb,
              filler=None, fill_points=()):
    """Causal attention for head h over q-chunks c_pair.

    Softmax denominator: exp tiles pair-summed on DVE into a bf16
    accumulator; one ones-matmul per (head, chunk) broadcasts the
    partition-sum. filler() emits one independent PE work unit at up to 4
    insertion points to bridge exp-latency stalls.
    """
    def fill(point):
        if filler is not None and point in fill_points:
            filler()

    accs, ctxps = [], []
    for ci, c in enumerate(c_pair):
        acc = accp.tile([P, QC], BF16, tag=f"acc{ci}")
        ctx_ps = psC.tile([P, QC], F32, tag=f"ctx{ci}")
        qs = qt_sb[:, h, c * QC:(c + 1) * QC]
        nd = 4 * c  # number of full (non-diagonal) k-tiles
        for kt2 in range(0, nd, 2):
            sa = psS.tile([P, QC], F32, tag="s")
            nc.tensor.matmul(sa, kt_sb[:, h, kt2 * P:(kt2 + 1) * P], qs,
                             start=True, stop=True)
            sb_ = psS.tile([P, QC], F32, tag="s")
            nc.tensor.matmul(sb_, kt_sb[:, h, (kt2 + 1) * P:(kt2 + 2) * P],
                             qs, start=True, stop=True)
            p_a = pp.tile([P, QC], BF16, tag="p")
            nc.scalar.activation(p_a, sa, EXP, scale=SCALE)
            p_b = pp.tile([P, QC], BF16, tag="p")
            nc.scalar.activation(p_b, sb_, EXP, scale=SCALE)
            if kt2 == 0:
                nc.vector.tensor_add(acc, p_a, p_b)
            else:
                pr = prp.tile([P, QC], BF16, tag="pr")
                nc.vector.tensor_add(pr, p_a, p_b)
                nc.vector.tensor_add(acc, acc, pr)
            nc.tensor.matmul(ctx_ps, vt_sb[:, kt2, h], p_a,
                             start=(kt2 == 0), stop=False)
            nc.tensor.matmul(ctx_ps, vt_sb[:, kt2 + 1, h], p_b,
                             start=False, stop=False)
        if ci == 1:
            fill(1)
        for j in range(4):
            F = QC - j * P
            kt = nd + j
            s1 = psS.tile([P, QC], F32, tag="s")
            nc.tensor.matmul(s1[:, 0:F], kt_sb[:, h, kt * P:(kt + 1) * P],
                             qs[:, j * P:QC], start=True, stop=True)
            nc.vector.tensor_add(s1[:, 0:P], s1[:, 0:P], mask_sb)
            p1 = pp.tile([P, QC], BF16, tag="p")
            nc.scalar.activation(p1[:, 0:F], s1[:, 0:F], EXP, scale=SCALE)
            if j == 0 and nd == 0:
                nc.vector.tensor_copy(acc, p1)
            else:
                nc.vector.tensor_add(acc[:, j * P:QC], acc[:, j * P:QC],
                                     p1[:, 0:F])
            nc.tensor.matmul(ctx_ps[:, j * P:QC], vt_sb[:, kt, h], p1[:, 0:F],
                             start=(nd == 0 and j == 0), stop=(j == 3))
        accs.append(acc)
        ctxps.append(ctx_ps)
        fill(0 if ci == 0 else 2)
    for ci in range(2):
        zw = psZ.tile([P, QC], F32, tag="z")
        nc.tensor.matmul(zw, ones_sb, accs[ci], start=True, stop=True)
        iz = izp.tile([P, QC], F32, tag=f"iz{ci}")
        nc.vector.reciprocal_approx_fast(iz, zw)
        nc.vector.tensor_mul(ctx2[:, ci, h], ctxps[ci], iz)
    fill(3)


_NC = None
DEBUG_NC = False


def _get_nc():
    global _NC
    if _NC is None:
        _NC = build_kernel(debug_dump=DEBUG_NC)
    return _NC


def _make_mask():
    m = np.zeros((P, P), dtype=np.float32)
    i = np.arange(P)[:, None]
    col = np.arange(P)[None, :]
    m[i > col] = NEG
    return m


def kernel(x, Wq, Wk, Wv, Wo, _trace=False, _trace_kwargs=None):
    bf16 = ml_dtypes.bfloat16
    x = np.asarray(x, dtype=np.float32)
    Wq = np.asarray(Wq, dtype=np.float32)
    Wk = np.asarray(Wk, dtype=np.float32)
    Wv = np.asarray(Wv, dtype=np.float32)
    Wo = np.asarray(Wo, dtype=np.float32)

    nc = _get_nc()
    mask = _make_mask()

    # [d_out, d_in] -> [h, p, ko, dd] tiles per head-group chunk of 8 heads
    def tile_qk(W, g):
        wt = W.T[:, g * GD:(g + 1) * GD]              # [D, GD]
        return np.ascontiguousarray(
            wt.reshape(NDK, P, G, HD).transpose(2, 1, 0, 3).astype(bf16))

    def tile_v(W, g):
        wt = W.T[:, g * GD:(g + 1) * GD]              # [D, GD]
        return np.ascontiguousarray(
            wt.reshape(NDK, P, 2, QC).transpose(2, 1, 0, 3).astype(bf16))

    def tile_wo(W, g):
        wt = W.T[g * GD:(g + 1) * GD, :]              # [GD, D]
        # [nt, p(of head block), hh, 128]
        return np.ascontiguousarray(
            wt.reshape(G, P, NDK, P).transpose(2, 1, 0, 3).astype(bf16))

    in_maps = []
    for core in range(8):
        b, g = divmod(core, 2)
        in_maps.append({
            "xT": np.ascontiguousarray(x[b].T.astype(bf16)),
            "wqT": tile_qk(Wq, g),
            "wkT": tile_qk(Wk, g),
            "wvT": tile_v(Wv, g),
            "woT": tile_wo(Wo, g),
            "maskadd": mask,
        })

    kwargs = {}
    if _trace:
        kwargs.update(trace=True, **(_trace_kwargs or {}))
    res = run_bass_kernel_spmd(nc, in_maps, core_ids=list(range(8)), **kwargs)

    out = np.empty((B, T, D), dtype=np.float32)
    for b in range(B):
        acc = (np.asarray(res.results[2 * b]["outT"], dtype=np.float32)
               + np.asarray(res.results[2 * b + 1]["outT"], dtype=np.float32))
        out[b] = acc.T
    if _trace:
        return out, res
    return out


# revision 41
# speedup vs baseline: 1.0624x; 1.0054x over previous
"""Multi-head causal attention on 8 Trainium2 cores.

Sharding: core = (batch b in 0..3, head-group g in 0..1). Each core computes
Q/K/V projections for its 8 heads of its batch, causal attention, and a
partial output projection (Wo row-split); host sums the two partials per
batch and transposes back.

Device layout notes (v3 — bf16 SBUF-resident, phase-overlapped):
  - All matmul inputs are bf16 (1 cyc/row on PE, same as fp32r, half SBUF).
  - Q^T, K^T, V stay resident in SBUF between projection and attention.
  - A1 projects Q/K with k-outer 8-bank accumulation so PE streams with the
    x DMA; Q and K chunk groups are staggered so PSUM copies never stall.
  - h6/h7's chunk-2,3 projections are deferred and interleaved into half-0
    attention as PE filler (covers the exp-latency ladder + keeps the PE
    pstate at max clock).
  - Softmax denominator via DVE pair-sums + one ones-matmul per head.
  - Half-0's output projection is interleaved inside half-1's attention kt
    loops; wo is loaded in nt-slices matching consumption order.
"""

import numpy as np
import ml_dtypes

import concourse.bacc as bacc
import concourse.mybir as mybir
import concourse.tile as tile
from concourse.bass_utils import run_bass_kernel_spmd

B, T, D = 4, 2048, 2048
NH, HD = 16, 128
G = 8                       # heads per core
GD = G * HD                 # 1024, group channel width
P = 128
QC = 512                    # q-chunk (PSUM bank width in fp32)
NKT = T // P                # 16 k-tiles over the sequence
NDK = D // P                # 16 k-tiles over d_in
NQC = T // QC               # 4 q-chunks
SCALE = 1.0 / float(np.sqrt(HD))
NEG = -1.0e30

F32 = mybir.dt.float32
F32R = mybir.dt.float32r
BF16 = mybir.dt.bfloat16
EXP = mybir.ActivationFunctionType.Exp


DEFER = True
A1_STREAM = True


def build_kernel(debug_dump=False):
    nc = bacc.Bacc("TRN2", target_bir_lowering=False, debug=False, num_devices=8,
                   dynamic_dma_scratch_size=2048)

    xT = nc.dram_tensor("xT", [D, T], BF16, kind="ExternalInput")
    # pre-tiled on host: wq/wk [head, p, ko, d], wv [dchunk, p, ko, c]
    wqT = nc.dram_tensor("wqT", [G, P, NDK, HD], BF16, kind="ExternalInput")
    wkT = nc.dram_tensor("wkT", [G, P, NDK, HD], BF16, kind="ExternalInput")
    wvT = nc.dram_tensor("wvT", [2, P, NDK, QC], BF16, kind="ExternalInput")
    # wo pre-tiled nt-major: [nt, p, hh, 128]
    woT = nc.dram_tensor("woT", [NDK, P, G, P], BF16, kind="ExternalInput")
    # triangle mask: NEG where partition (k) > column (q) within a 128 block
    maskadd = nc.dram_tensor("maskadd", [P, P], F32, kind="ExternalInput")
    outT = nc.dram_tensor("outT", [D, T], BF16, kind="ExternalOutput")
    if debug_dump:
        qtD = nc.dram_tensor("qtD", [P, G, T], BF16, kind="ExternalOutput")
        ktD = nc.dram_tensor("ktD", [P, G, T], BF16, kind="ExternalOutput")
        vtD = nc.dram_tensor("vtD", [P, NKT, G, HD], BF16, kind="ExternalOutput")
        c2D = nc.dram_tensor("c2D", [P, 2, G, QC], BF16, kind="ExternalOutput")
        mkD = nc.dram_tensor("mkD", [P, P], F32, kind="ExternalOutput")
        onD = nc.dram_tensor("onD", [P, P], F32, kind="ExternalOutput")

    xT_t = xT.rearrange("(ko p) t -> p ko t", p=P)
    outT_t = outT.rearrange("(no p) t -> p no t", p=P)

    with tile.TileContext(nc) as tc:
        with (
            tc.tile_pool(name="const", bufs=1) as constp,
            tc.tile_pool(name="kvq", bufs=1) as kvqp,
            tc.tile_pool(name="c2p0", bufs=1) as c2p0,
        ):
            ones_sb = constp.tile([P, P], BF16)
            nc.vector.memset(ones_sb, 1.0)
            mask_sb = constp.tile([P, P], F32)

            kt_sb = kvqp.tile([P, G, T], BF16)           # K^T per head
            qt_sb = kvqp.tile([P, G, T], BF16)           # Q^T per head
            vt_sb = kvqp.tile([P, NKT, G, HD], BF16)     # V per head
            ctx2_0 = c2p0.tile([P, 2, G, QC], BF16)      # half-0 attn output

            outq = [nc.sync, nc.sync]
            oidx = [0]

            def qk_copy(dst, h, c, ps, eng):
                if eng == 0:
                    nc.scalar.copy(dst[:, h, c * QC:(c + 1) * QC], ps)
                else:
                    nc.vector.tensor_copy(dst[:, h, c * QC:(c + 1) * QC], ps)

            with tc.tile_pool(name="xpool", bufs=1) as xpool:
                xt_sb = xpool.tile([P, NDK, T], BF16)    # 8 MB, resident

                # ---------------- A1: Q/K projections ----------------
                with (
                    tc.tile_pool(name="w1pool", bufs=2) as w1p,
                    tc.tile_pool(name="psA1", bufs=1, space="PSUM") as psA1,
                ):
                    # first head's weights before the x stream; the first
                    # few k-slices land as small DMAs so matmul k=0 can
                    # start as early as possible
                    wq_sb = w1p.tile([P, NDK, HD], BF16, tag="wq")
                    for kk in (slice(0, 2), slice(2, 8), slice(8, NDK)):
                        nc.scalar.dma_start(wq_sb[:, kk], wqT[0, :, kk])
                    wk_sb = w1p.tile([P, NDK, HD], BF16, tag="wk")
                    for kk in (slice(0, 2), slice(2, 8), slice(8, NDK)):
                        nc.scalar.dma_start(wk_sb[:, kk], wkT[0, :, kk])
                    nc.scalar.dma_start(mask_sb, maskadd[:])

                    for k in range(NDK):
                        if k < 2:
                            for cc in range(4):
                                nc.sync.dma_start(
                                    xt_sb[:, k, cc * QC:(cc + 1) * QC],
                                    xT_t[:, k, cc * QC:(cc + 1) * QC])
                        else:
                            nc.sync.dma_start(xt_sb[:, k], xT_t[:, k])

                    for h in range(G):
                        full = h < 6
                        if h > 0:
                            wq_sb = w1p.tile([P, NDK, HD], BF16, tag="wq")
                            nc.scalar.dma_start(wq_sb, wqT[h])
                            wk_sb = w1p.tile([P, NDK, HD], BF16, tag="wk")
                            nc.scalar.dma_start(wk_sb, wkT[h])
                        # group 1: Q all chunks (c01 for h6/h7) + K c0,c1.
                        # h0 takes all 8 banks so PE stays saturated while
                        # the x slices stream in
                        qcs = (0, 1, 2, 3) if full else (0, 1)
                        kcs = (0, 1, 2, 3) if h == 0 else (0, 1)
                        psq = {c: psA1.tile([P, QC], F32, tag=f"q{c}", name=f"psq{c}")
                               for c in qcs}
                        psk = {c: psA1.tile([P, QC], F32, tag=f"k{c}", name=f"psk{c}")
                               for c in kcs}
                        for k in range(NDK):
                            st, sp = (k == 0), (k == NDK - 1)
                            for c in qcs:
                                nc.tensor.matmul(
                                    psq[c], wq_sb[:, k],
                                    xt_sb[:, k, c * QC:(c + 1) * QC],
                                    start=st, stop=sp)
                            for c in kcs:
                                nc.tensor.matmul(
                                    psk[c], wk_sb[:, k],
                                    xt_sb[:, k, c * QC:(c + 1) * QC],
                                    start=st, stop=sp)
                        for i, c in enumerate(qcs):
                            qk_copy(qt_sb, h, c, psq[c], i % 2)
                        for i, c in enumerate(kcs):
                            qk_copy(kt_sb, h, c, psk[c], (i + 1) % 2)
                        # group 2: K c2,c3 (full heads) — drains while group-1
                        # copies free their banks
                        if full and h != 0:
                            psk2 = {c: psA1.tile([P, QC], F32, tag=f"k{c}", name=f"psk2{c}")
                                    for c in (2, 3)}
                            for k in range(NDK):
                                for c in (2, 3):
                                    nc.tensor.matmul(
                                        psk2[c], wk_sb[:, k],
                                        xt_sb[:, k, c * QC:(c + 1) * QC],
                                        start=(k == 0), stop=(k == NDK - 1))
                            qk_copy(kt_sb, h, 2, psk2[2], 0)
                            qk_copy(kt_sb, h, 3, psk2[3], 1)

                # ---------------- A2: V projection (dc-split) ----------------
                # wv streams per-k so the V k-loop starts early; two deferred
                # h6 c2/c3 projections fill the PE while the first slices land
                for dc in range(2):
                    with (
                        tc.tile_pool(name=f"wv{dc}", bufs=1) as wvp,
                        tc.tile_pool(name=f"wA2{dc}", bufs=1) as wA2p,
                        tc.tile_pool(name=f"psV{dc}", bufs=2,
                                     space="PSUM") as psV,
                    ):
                        wv_sb = wvp.tile([P, NDK, QC], BF16)
                        for k0 in range(0, NDK, 4):
                            nc.scalar.dma_start(wv_sb[:, k0:k0 + 4],
                                                wvT[dc, :, k0:k0 + 4])
                        wA2 = wA2p.tile([P, NDK, HD], BF16)
                        nc.sync.dma_start(wA2, (wqT if dc == 0 else wkT)[6])
                        for c in (2, 3):
                            ps = psV.tile([P, QC], F32, tag="def", bufs=1)
                            for k in range(NDK):
                                nc.tensor.matmul(
                                    ps, wA2[:, k],
                                    xt_sb[:, k, c * QC:(c + 1) * QC],
                                    start=(k == 0), stop=(k == NDK - 1))
                            dst = qt_sb if dc == 0 else kt_sb
                            nc.vector.tensor_copy(
                                dst[:, 6, c * QC:(c + 1) * QC], ps)
                        for ts in range(NKT):
                            ps = psV.tile([P, QC], F32, tag="v")
                            for k in range(NDK):
                                nc.tensor.matmul(
                                    ps, xt_sb[:, k, ts * P:(ts + 1) * P],
                                    wv_sb[:, k],
                                    start=(k == 0), stop=(k == NDK - 1))
                            nc.vector.tensor_copy(
                                vt_sb[:, ts, 4 * dc:4 * (dc + 1), :],
                                ps.rearrange("p (g c) -> p g c", g=4))

                # ---------------- overlap: half-0 attention + deferred
                # c2/c3 projections of h6/h7 as PE filler ----------------
                with (
                    tc.tile_pool(name="w2pool", bufs=2) as w2p,
                    tc.tile_pool(name="pp0", bufs=6) as pp0,
                    tc.tile_pool(name="prp0", bufs=2) as prp0,
                    tc.tile_pool(name="accp0", bufs=1) as accp0,
                    tc.tile_pool(name="izp0", bufs=1) as izp0,
                    tc.tile_pool(name="psS0", bufs=4, space="PSUM") as psS0,
                    tc.tile_pool(name="psC0", bufs=1, space="PSUM") as psC0,
                    tc.tile_pool(name="psZD", bufs=1, space="PSUM") as psZD,
                ):
                    # deferred unit list: grouped so one w tile serves 2 units
                    defer = [(wt, 7, c) for wt in (0, 1) for c in (2, 3)]
                    dstate = {"i": 0, "w": None}

                    def defer_w_load(gi):
                        wt, h, _ = defer[2 * gi]
                        w = w2p.tile([P, NDK, HD], BF16, tag="w2")
                        nc.scalar.dma_start(w, (wqT if wt == 0 else wkT)[h])
                        return w

                    dstate["w"] = defer_w_load(0)
                    dstate["wnext"] = None

                    def filler0():
                        i = dstate["i"]
                        if i >= len(defer):
                            return
                        wt, h, c = defer[i]
                        if i % 2 == 0 and i + 2 < len(defer):
                            dstate["wnext"] = defer_w_load((i + 2) // 2)
                        w_sb = dstate["w"]
                        ps = psZD.tile([P, QC], F32, tag="def")
                        for k in range(NDK):
                            nc.tensor.matmul(
                                ps, w_sb[:, k],
                                xt_sb[:, k, c * QC:(c + 1) * QC],
                                start=(k == 0), stop=(k == NDK - 1))
                        dst = qt_sb if wt == 0 else kt_sb
                        nc.vector.tensor_copy(
                            dst[:, h, c * QC:(c + 1) * QC], ps)
                        if i % 2 == 1:
                            dstate["w"] = dstate["wnext"]
                        dstate["i"] = i + 1

                    for h in range(G):
                        attn_head(nc, h, (0, 1), ctx2_0, psC0, psS0, psZD,
                                  pp0, prp0, accp0, izp0,
                                  kt_sb, qt_sb, vt_sb, ones_sb, mask_sb,
                                  filler=filler0,
                                  fill_points=(1,) if h % 2 == 0 else ())

            if debug_dump:
                nc.sync.dma_start(mkD[:], mask_sb)
                nc.sync.dma_start(onD[:], ones_sb.bitcast(F32))
                for h_ in range(G):
                    nc.sync.dma_start(qtD[:, h_], qt_sb[:, h_])
                    nc.sync.dma_start(ktD[:, h_], kt_sb[:, h_])
                    nc.sync.dma_start(c2D[:, 0, h_], ctx2_0[:, 0, h_])
                    nc.sync.dma_start(c2D[:, 1, h_], ctx2_0[:, 1, h_])
                for ts_ in range(NKT):
                    nc.gpsimd.dma_start(vtD[:, ts_], vt_sb[:, ts_])

            # x freed; half-1 attention + both output projections
            with (
                tc.tile_pool(name="wopool", bufs=1) as wop,
                tc.tile_pool(name="c2p1", bufs=1) as c2p1,
                tc.tile_pool(name="pp1", bufs=6) as pp1,
                tc.tile_pool(name="prp1", bufs=2) as prp1,
                tc.tile_pool(name="accp1", bufs=1) as accp1,
                tc.tile_pool(name="izp1", bufs=1) as izp1,
                tc.tile_pool(name="opool", bufs=3) as op_,
                tc.tile_pool(name="psS1", bufs=4, space="PSUM") as psS1,
                tc.tile_pool(name="psC1", bufs=1, space="PSUM") as psC1,
                tc.tile_pool(name="psZO", bufs=1, space="PSUM") as psZO,
            ):
                ctx2_1 = c2p1.tile([P, 2, G, QC], BF16)
                wo_sb = wop.tile([P, NDK, G, P], BF16)   # 4 MB, nt-major
                for nt in range(NDK):
                    outq[nt % 2].dma_start(wo_sb[:, nt], woT[nt])

                ostate = {"i": 0}
                otiles = [(nt, ci, ci, ctx2_0) for nt in range(NDK)
                          for ci in range(2)]

                def outproj_tile(nt, ci, c, ctx2src, final=False):
                    if final:
                        o_ps = psC1.tile([P, QC], F32,
                                         tag=f"ctx{oidx[0] % 2}",
                                         name="o_ps")
                    else:
                        o_ps = psZO.tile([P, QC], F32, tag="o")
                    for hh in range(G):
                        nc.tensor.matmul(
                            o_ps, wo_sb[:, nt, hh], ctx2src[:, ci, hh],
                            start=(hh == 0), stop=(hh == G - 1))
                    o_sb = op_.tile([P, QC], BF16, tag="osb")
                    if oidx[0] % 2 == 0:
                        nc.scalar.copy(o_sb, o_ps)
                    else:
                        nc.vector.tensor_copy(o_sb, o_ps)
                    nc.sync.dma_start(
                        outT_t[:, nt, c * QC:(c + 1) * QC], o_sb)
                    oidx[0] += 1

                def filler1():
                    i = ostate["i"]
                    if i >= len(otiles):
                        return
                    outproj_tile(*otiles[i])
                    ostate["i"] = i + 1

                # two outproj tiles up front to cover the phase
                # transition before head 0's exp ladder warms up
                filler1()
                filler1()
                for h in range(G):
                    attn_head(nc, h, (2, 3), ctx2_1, psC1, psS1, psZO,
                              pp1, prp1, accp1, izp1,
                              kt_sb, qt_sb, vt_sb, ones_sb, mask_sb,
                              filler=filler1, fill_points=(0, 1, 2, 3))
                for nt in range(NDK):
                    for ci in range(2):
                        outproj_tile(nt, ci, 2 + ci, ctx2_1, final=True)

    nc.finalize()
    return nc


def attn_head(nc, h, c_pair, ctx2, psC, psS, psZ, pp, prp, accp, izp,
              kt_sb, qt_sb, vt_sb, ones_sb, mask_sb,
              filler=None, fill_points=()):
    """Causal attention for head h over q-chunks c_pair.

    Softmax denominator: exp tiles pair-summed on DVE into a bf16
    accumulator; one ones-matmul per (head, chunk) broadcasts the
    partition-sum. filler() emits one independent PE work unit at up to 4
    insertion points to bridge exp-latency stalls.
    """
    def fill(point):
        if filler is not None and point in fill_points:
            filler()

    accs, ctxps = [], []
    for ci, c in enumerate(c_pair):
        acc = accp.tile([P, QC], BF16, tag=f"acc{ci}")
        ctx_ps = psC.tile([P, QC], F32, tag=f"ctx{ci}")
        qs = qt_sb[:, h, c * QC:(c + 1) * QC]
        nd = 4 * c  # number of full (non-diagonal) k-tiles
        for kt2 in range(0, nd, 2):
            sa = psS.tile([P, QC], F32, tag="s")
            nc.tensor.matmul(sa, kt_sb[:, h, kt2 * P:(kt2 + 1) * P], qs,
                             start=True, stop=True)
            sb_ = psS.tile([P, QC], F32, tag="s")
            nc.tensor.matmul(sb_, kt_sb[:, h, (kt2 + 1) * P:(kt2 + 2) * P],
                             qs, start=True, stop=True)
            p_a = pp.tile([P, QC], BF16, tag="p")
            nc.scalar.activation(p_a, sa, EXP, scale=SCALE)
            p_b = pp.tile([P, QC], BF16, tag="p")
            nc.scalar.activation(p_b, sb_, EXP, scale=SCALE)
            if kt2 == 0:
                nc.vector.tensor_add(acc, p_a, p_b)
            else:
                pr = prp.tile([P, QC], BF16, tag="pr")
                nc.vector.tensor_add(pr, p_a, p_b)
                nc.vector.tensor_add(acc, acc, pr)
            nc.tensor.matmul(ctx_ps, vt_sb[:, kt2, h], p_a,
                             start=(kt2 == 0), stop=False)
            nc.tensor.matmul(ctx_ps, vt_sb[:, kt2 + 1, h], p_b,
                             start=False, stop=False)
        if ci == 1:
            fill(1)
        for j in range(4):
            F = QC - j * P
            kt = nd + j
            s1 = psS.tile([P, QC], F32, tag="s")
            nc.tensor.matmul(s1[:, 0:F], kt_sb[:, h, kt * P:(kt + 1) * P],
                             qs[:, j * P:QC], start=True, stop=True)
            nc.vector.tensor_add(s1[:, 0:P], s1[:, 0:P], mask_sb)
            p1 = pp.tile([P, QC], BF16, tag="p")
            nc.scalar.activation(p1[:, 0:F], s1[:, 0:F], EXP, scale=SCALE)
            if j == 0 and nd == 0:
                nc.vector.tensor_copy(acc, p1)
            else:
                nc.vector.tensor_add(acc[:, j * P:QC], acc[:, j * P:QC],
                                     p1[:, 0:F])
            nc.tensor.matmul(ctx_ps[:, j * P:QC], vt_sb[:, kt, h], p1[:, 0:F],
                             start=(nd == 0 and j == 0), stop=(j == 3))
        accs.append(acc)
        ctxps.append(ctx_ps)
        fill(0 if ci == 0 else 2)
    for ci in range(2):
        zw = psZ.tile([P, QC], F32, tag="z")
        nc.tensor.matmul(zw, ones_sb, accs[ci], start=True, stop=True)
        iz = izp.tile([P, QC], F32, tag=f"iz{ci}")
        nc.vector.reciprocal_approx_fast(iz, zw)
        nc.vector.tensor_mul(ctx2[:, ci, h], ctxps[ci], iz)
    fill(3)


_NC = None
DEBUG_NC = False


def _get_nc():
    global _NC
    if _NC is None:
        _NC = build_kernel(debug_dump=DEBUG_NC)
    return _NC


def _make_mask():
    m = np.zeros((P, P), dtype=np.float32)
    i = np.arange(P)[:, None]
    col = np.arange(P)[None, :]
    m[i > col] = NEG
    return m


def kernel(x, Wq, Wk, Wv, Wo, _trace=False, _trace_kwargs=None):
    bf16 = ml_dtypes.bfloat16
    x = np.asarray(x, dtype=np.float32)
    Wq = np.asarray(Wq, dtype=np.float32)
    Wk = np.asarray(Wk, dtype=np.float32)
    Wv = np.asarray(Wv, dtype=np.float32)
    Wo = np.asarray(Wo, dtype=np.float32)

    nc = _get_nc()
    mask = _make_mask()

    # [d_out, d_in] -> [h, p, ko, dd] tiles per head-group chunk of 8 heads
    def tile_qk(W, g):
        wt = W.T[:, g * GD:(g + 1) * GD]              # [D, GD]
        return np.ascontiguousarray(
            wt.reshape(NDK, P, G, HD).transpose(2, 1, 0, 3).astype(bf16))

    def tile_v(W, g):
        wt = W.T[:, g * GD:(g + 1) * GD]              # [D, GD]
        return np.ascontiguousarray(
            wt.reshape(NDK, P, 2, QC).transpose(2, 1, 0, 3).astype(bf16))

    def tile_wo(W, g):
        wt = W.T[g * GD:(g + 1) * GD, :]              # [GD, D]
        # [nt, p(of head block), hh, 128]
        return np.ascontiguousarray(
            wt.reshape(G, P, NDK, P).transpose(2, 1, 0, 3).astype(bf16))

    in_maps = []
    for core in range(8):
        b, g = divmod(core, 2)
        in_maps.append({
            "xT": np.ascontiguousarray(x[b].T.astype(bf16)),
            "wqT": tile_qk(Wq, g),
            "wkT": tile_qk(Wk, g),
            "wvT": tile_v(Wv, g),
            "woT": tile_wo(Wo, g),
            "maskadd": mask,
        })

    kwargs = {}
    if _trace:
        kwargs.update(trace=True, **(_trace_kwargs or {}))
    res = run_bass_kernel_spmd(nc, in_maps, core_ids=list(range(8)), **kwargs)

    out = np.empty((B, T, D), dtype=np.float32)
    for b in range(B):
        acc = (np.asarray(res.results[2 * b]["outT"], dtype=np.float32)
               + np.asarray(res.results[2 * b + 1]["outT"], dtype=np.float32))
        out[b] = acc.T
    if _trace:
        return out, res
    return out
